# revision 30
# baseline (speedup 1.0000x reference)
"""Causal multi-head attention (B=4, T=2048, C=1024, 16 heads) on 8 TRN2 cores.

Megatron-style tensor parallel: 2 heads per core. QKV projection column-split,
output projection row-split; per-core partial outputs are summed on the host.

Device-side layout is feature-major (transposed): Q^T/K^T [128=(2 heads x 64), T],
attention computed as S^T = K @ Q^T in [j, i] layout so softmax'd probabilities
come out pre-transposed for the AV matmul (no on-chip P transposes). The softmax
denominator is folded into the AV matmul via a ones-column appended to V.
All matmuls run in float32r (TF32-like, full PE rate at N>=256).
"""
import numpy as np

import concourse.bass as bass
import concourse.mybir as mybir
import concourse.tile as tile
from concourse import bacc
from concourse.bass_utils import run_bass_kernel_spmd

F32 = mybir.dt.float32
F32R = mybir.dt.float32r
AF = mybir.ActivationFunctionType

N_CORES = 8
B, T, C = 4, 2048, 1024
HD = 64          # head dim
HPC = 2          # heads per core
TOK = B * T      # 8192 tokens
NKK = C // 128   # 8 contraction chunks for qkv projection
TB = 512         # token block (matmul moving N)
NTB = T // TB    # 4 token blocks per batch
NIB = T // 512   # 4 i-blocks per batch
NJT = T // 128   # 16 j-tiles per batch
SCALE = 1.0 / 8.0  # 1/sqrt(HD)

LAST_RESULTS = None  # test harness reads exec_time_ns from here
_NC_CACHE = None


def build_kernel(loop_n: int = 1):
    nc = bacc.Bacc("TRN2", target_bir_lowering=False, debug=False, num_devices=N_CORES)

    xT = nc.dram_tensor("xT", [C, TOK], F32, kind="ExternalInput")
    wqkv = nc.dram_tensor("wqkv", [C, 3 * 128], F32, kind="ExternalInput")
    bqkv = nc.dram_tensor("bqkv", [3 * 128, 1], F32, kind="ExternalInput")
    wp = nc.dram_tensor("wp", [128, C], F32, kind="ExternalInput")
    bp = nc.dram_tensor("bp", [C, 1], F32, kind="ExternalInput")
    mask_d = nc.dram_tensor("mask", [128, 128], F32, kind="ExternalInput")
    ident_d = nc.dram_tensor("ident2", [128, HD], F32, kind="ExternalInput")
    oT = nc.dram_tensor("oT", [C, TOK], F32, kind="ExternalOutput")

    with tile.TileContext(nc) as tc:
        with (
            tc.tile_pool(name="consts", bufs=1) as consts,
            tc.tile_pool(name="xf", bufs=4) as xfp,
            tc.tile_pool(name="qkvt", bufs=2) as qkvt,
            tc.tile_pool(name="vtok", bufs=2) as vtokp,
            tc.tile_pool(name="pexp", bufs=3) as pexpp,
            tc.tile_pool(name="ytp", bufs=2) as ytp,
            tc.tile_pool(name="otp", bufs=3) as otp,
            tc.tile_pool(name="small", bufs=1) as small,
            tc.tile_pool(name="ps1", bufs=2, space="PSUM") as ps1,
            tc.tile_pool(name="ps2", bufs=2, space="PSUM") as ps2,
            tc.tile_pool(name="ps3", bufs=2, space="PSUM") as ps3,
        ):
            import contextlib

            loop_cm = tc.For_i(0, loop_n, 1) if loop_n > 1 else contextlib.nullcontext()
            # ---- constants ----
            # qkv weights: [C, 384] -> sbuf [128, NKK*384] float32r
            w_sb = consts.tile([128, NKK, 3 * 128], F32R, tag="w_sb")
            wq = wqkv.rearrange("(kk p) m -> kk p m", p=128)
            for kk in range(NKK):
                st = small.tile([128, 3 * 128], F32, tag="wst", name="wst")
                nc.sync.dma_start(out=st, in_=wq[kk])
                nc.vector.tensor_copy(out=w_sb[:, kk, :], in_=st)

            # proj weights [128, C] -> f32r (staged through the same slot)
            wp_sb = consts.tile([128, C], F32R, tag="wp_sb")
            wpr = wp.rearrange("p (hc c) -> p hc c", hc=2)
            for hc in range(2):
                st = small.tile([128, 2 * 256], F32, tag="wst", name="wst")
                nc.sync.dma_start(out=st, in_=wpr[:, hc, :])
                nc.vector.tensor_copy(out=wp_sb[:, 512 * hc : 512 * (hc + 1)], in_=st)

            # biases
            bqkv_sb = consts.tile([128, 3], F32, tag="bqkv_sb")
            bq3 = bqkv.rearrange("(g p) one -> g p one", p=128)
            for g in range(3):
                nc.sync.dma_start(out=bqkv_sb[:, g : g + 1], in_=bq3[g])
            bp_sb = consts.tile([128, 8], F32, tag="bp_sb")
            bp8 = bp.rearrange("(oc p) one -> oc p one", p=128)
            for oc in range(8):
                nc.sync.dma_start(out=bp_sb[:, oc : oc + 1], in_=bp8[oc])

            # causal mask tile (fp32 is fine: only a DVE-mul operand)
            mask_sb = consts.tile([128, 128], F32, tag="mask_sb")
            nc.sync.dma_start(out=mask_sb, in_=mask_d[:, :])

            # ones column (f32r, DVE-rounded)
            ones_f32 = consts.tile([128, 1], F32, tag="ones_f32")
            nc.vector.memset(ones_f32, 1.0)
            ones_r = consts.tile([128, 1], F32R, tag="ones_r")
            nc.vector.tensor_copy(out=ones_r, in_=ones_f32)

            # stacked identity (I64 twice) for PE transpose, f32r
            ident_f32 = consts.tile([128, HD], F32, tag="ident_f32")
            nc.sync.dma_start(out=ident_f32, in_=ident_d[:, :])
            ident_r = consts.tile([128, HD], F32R, tag="ident_r")
            nc.vector.tensor_copy(out=ident_r, in_=ident_f32)

            def emit_proj_chunk(yt_ib, b_p, tb):
                t0p = b_p * T
                for oc in range(8):
                    po = ps1.tile([128, TB], F32, tag="mm512", name="po")
                    nc.tensor.matmul(
                        po,
                        wp_sb[:, 128 * oc : 128 * (oc + 1)],
                        yt_ib,
                        start=True,
                        stop=True,
                    )
                    ot = otp.tile([128, TB], F32, tag="ot", name="ot")
                    if (oc + tb) % 2 == 0:
                        nc.vector.tensor_scalar_add(
                            out=ot, in0=po, scalar1=bp_sb[:, oc : oc + 1]
                        )
                    else:
                        nc.scalar.activation(
                            out=ot, in_=po, func=AF.Identity,
                            bias=bp_sb[:, oc : oc + 1],
                        )
                    nc.sync.dma_start(
                        out=oT[128 * oc : 128 * (oc + 1), t0p + TB * tb : t0p + TB * (tb + 1)],
                        in_=ot,
                    )

            xcache = {}

            def emit_x_load(bb, tb):
                tt0 = bb * T
                xrs = []
                for kk in range(NKK):
                    stg = xfp.tile([128, TB], F32, tag="xst", name="xst", bufs=6)
                    nc.sync.dma_start(
                        out=stg,
                        in_=xT[128 * kk : 128 * (kk + 1), tt0 + TB * tb : tt0 + TB * (tb + 1)],
                    )
                    xr = xfp.tile([128, TB], F32R, tag=f"xr{kk}", name=f"xr{kk}")
                    eng = nc.gpsimd if kk % 2 == 0 else nc.vector
                    eng.tensor_copy(out=xr, in_=stg)
                    xrs.append(xr)
                return xrs

            pending = []
            with loop_cm:
              for b in range(B):
                t0 = b * T  # global token offset of this batch

                # ---- phase A: QKV^T for batch b ----
                qt = qkvt.tile([128, T], F32R, tag="qt")
                kt = qkvt.tile([128, T], F32R, tag="kt")
                vt = qkvt.tile([128, T], F32R, tag="vt", bufs=1)
                dsts = (qt, kt, vt)
                for tb in range(NTB):
                    xrs = xcache.pop((b, tb), None)
                    if xrs is None:
                        xrs = emit_x_load(b, tb)
                    for g in range(3):
                        pqkv = ps1.tile([128, TB], F32, tag="mm512")
                        for kk in range(NKK):
                            nc.tensor.matmul(
                                pqkv,
                                w_sb[:, kk, 128 * g : 128 * (g + 1)],
                                xrs[kk],
                                start=(kk == 0),
                                stop=(kk == NKK - 1),
                            )
                        nc.vector.tensor_scalar_add(
                            out=dsts[g][:, TB * tb : TB * (tb + 1)],
                            in0=pqkv,
                            scalar1=bqkv_sb[:, g : g + 1],
                        )

                # ---- phase B: V token-major [128 tok, 65] tiles ----
                vtok = [vtokp.tile([128, NJT, HD + 1], F32R, tag=f"vtok{h}", name=f"vtok{h}") for h in range(HPC)]
                for h in range(HPC):
                    ones_b = bass.AP(
                        tensor=ones_r.tensor,
                        offset=ones_r.offset,
                        ap=[ones_r.ap[0], [0, NJT], [1, 1]],
                    )
                    nc.vector.tensor_copy(out=vtok[h][:, :, HD : HD + 1], in_=ones_b)
                    for tt in range(NJT):
                        pvt = ps1.tile([128, HD], F32R, tag="mm512")
                        nc.tensor.transpose(
                            pvt,
                            vt[HD * h : HD * (h + 1), 128 * tt : 128 * (tt + 1)],
                            ident_r[HD * h : HD * (h + 1), :],
                        )
                        nc.vector.tensor_copy(out=vtok[h][:, tt, 0:HD], in_=pvt)

                # ---- phase C: causal attention ----
                for ib in range(NIB):
                    yt = ytp.tile([128, TB], F32R, tag=f"yt{ib}", name=f"yt{ib}", bufs=3)
                    if len(pending) >= 3:
                        emit_proj_chunk(*pending.pop(0))
                    if b + 1 < B and (b + 1, ib) not in xcache:
                        xcache[(b + 1, ib)] = emit_x_load(b + 1, ib)
                    i_lo = 512 * ib
                    pys = [ps3.tile([HD + 1, 512], F32, tag="y", name=f"py{hh}") for hh in range(HPC)]
                    njt = 4 * ib + 4
                    for jt in range(njt):
                        coff = max(0, 128 * jt - i_lo)
                        n = 512 - coff
                        pst = ps2.tile([128, 1024], F32, tag="s")
                        for h in range(HPC):
                            nc.tensor.matmul(
                                pst[:, 512 * h + coff : 512 * h + 512],
                                kt[HD * h : HD * (h + 1), 128 * jt : 128 * (jt + 1)],
                                qt[HD * h : HD * (h + 1), i_lo + coff : i_lo + 512],
                                start=True,
                                stop=True,
                                tile_position=(HD * h, 0),
                            )
                        pe = pexpp.tile([128, 1024], F32R, tag="pexp")
                        if coff == 0:
                            nc.scalar.activation(
                                out=pe[:, 0:1024], in_=pst[:, 0:1024], func=AF.Exp, scale=SCALE
                            )
                        else:
                            for h in range(HPC):
                                nc.scalar.activation(
                                    out=pe[:, 512 * h + coff : 512 * h + 512],
                                    in_=pst[:, 512 * h + coff : 512 * h + 512],
                                    func=AF.Exp,
                                    scale=SCALE,
                                )
                        if 128 * jt >= i_lo:  # diagonal tile: triangular mask
                            for h in range(HPC):
                                lo = 512 * h + coff
                                nc.vector.tensor_mul(
                                    pe[:, lo : lo + 128],
                                    pe[:, lo : lo + 128],
                                    mask_sb,
                                )
                        for h in range(HPC):
                            nc.tensor.matmul(
                                pys[h][:, coff:512],
                                vtok[h][:, jt, :],
                                pe[:, 512 * h + coff : 512 * h + 512],
                                start=(jt == 0),
                                stop=(jt == njt - 1),
                            )
                    for h in range(HPC):
                        rs = small.tile([1, 512], F32, tag=f"rs{h}", name=f"rs{h}")
                        nc.vector.reciprocal(out=rs, in_=pys[h][HD : HD + 1, :])
                        rb = small.tile([HD, 512], F32, tag=f"rb{h}", name=f"rb{h}")
                        nc.gpsimd.partition_broadcast(rb, rs)
                        nc.vector.tensor_mul(
                            yt[HD * h : HD * (h + 1), :],
                            pys[h][0:HD, :],
                            rb,
                        )
                    pending.append((yt, b, ib))
              for c in pending:
                  emit_proj_chunk(*c)
              pending.clear()

    nc.compile()
    return nc


def _get_nc():
    global _NC_CACHE
    if _NC_CACHE is None:
        _NC_CACHE = build_kernel()
    return _NC_CACHE


def make_in_maps(x, W_attn, b_attn, W_proj, b_proj):
    x = np.asarray(x, dtype=np.float32)
    W_attn = np.asarray(W_attn, dtype=np.float32)
    b_attn = np.asarray(b_attn, dtype=np.float32)
    W_proj = np.asarray(W_proj, dtype=np.float32)
    b_proj = np.asarray(b_proj, dtype=np.float32)

    xT = np.ascontiguousarray(x.reshape(TOK, C).T)
    mask = np.triu(np.ones((128, 128), dtype=np.float32))  # keep i >= j
    ident2 = np.concatenate([np.eye(HD, dtype=np.float32)] * 2, axis=0)

    in_maps = []
    for c in range(N_CORES):
        h0 = HPC * c
        qs = slice(HD * h0, HD * h0 + 128)
        ks = slice(C + HD * h0, C + HD * h0 + 128)
        vs = slice(2 * C + HD * h0, 2 * C + HD * h0 + 128)
        wqkv_c = np.ascontiguousarray(
            np.concatenate([W_attn[:, qs], W_attn[:, ks], W_attn[:, vs]], axis=1)
        )
        bqkv_c = np.ascontiguousarray(
            np.concatenate([b_attn[qs], b_attn[ks], b_attn[vs]])[:, None]
        )
        wp_c = np.ascontiguousarray(W_proj[128 * c : 128 * (c + 1), :])
        bp_c = (
            b_proj[:, None].astype(np.float32)
            if c == 0
            else np.zeros((C, 1), np.float32)
        )
        in_maps.append(
            {
                "xT": xT,
                "wqkv": wqkv_c,
                "bqkv": bqkv_c,
                "wp": wp_c,
                "bp": np.ascontiguousarray(bp_c),
                "mask": mask,
                "ident2": ident2,
            }
        )
    return in_maps


def gather(res):
    acc = np.zeros((C, TOK), dtype=np.float64)
    for r in res.results:
        acc += r["oT"].astype(np.float64)
    return acc.T.astype(np.float32).reshape(B, T, C)


def kernel(x, W_attn, b_attn, W_proj, b_proj):
    global LAST_RESULTS
    in_maps = make_in_maps(x, W_attn, b_attn, W_proj, b_proj)
    nc = _get_nc()
    res = run_bass_kernel_spmd(nc, in_maps, core_ids=list(range(N_CORES)))
    LAST_RESULTS = res
    return gather(res)


# revision 31
# speedup vs baseline: 1.1180x; 1.1180x over previous
"""Causal multi-head attention (B=4, T=2048, C=1024, 16 heads) on 8 TRN2 cores.

Megatron-style tensor parallel: 2 heads per core. QKV projection column-split,
output projection row-split; per-core partial outputs are summed on the host.

Device-side layout is feature-major (transposed): Q^T/K^T [128=(2 heads x 64), T],
attention computed as S^T = K @ Q^T in [j, i] layout so softmax'd probabilities
come out pre-transposed for the AV matmul (no on-chip P transposes). The softmax
denominator is folded into the AV matmul via a ones-column appended to V.
All matmuls run in float32r (TF32-like, full PE rate at N>=256).
"""
import numpy as np

import concourse.bass as bass
import concourse.mybir as mybir
import concourse.tile as tile
from concourse import bacc
from concourse.bass_utils import run_bass_kernel_spmd

F32 = mybir.dt.float32
F32R = mybir.dt.float32r
AF = mybir.ActivationFunctionType

N_CORES = 8
B, T, C = 4, 2048, 1024
HD = 64          # head dim
HPC = 2          # heads per core
TOK = B * T      # 8192 tokens
NKK = C // 128   # 8 contraction chunks for qkv projection
TB = 512         # token block (matmul moving N)
NTB = T // TB    # 4 token blocks per batch
NIB = T // 512   # 4 i-blocks per batch
NJT = T // 128   # 16 j-tiles per batch
SCALE = 1.0 / 8.0  # 1/sqrt(HD)

LAST_RESULTS = None  # test harness reads exec_time_ns from here
_NC_CACHE = None


def build_kernel(loop_n: int = 1):
    nc = bacc.Bacc("TRN2", target_bir_lowering=False, debug=False, num_devices=N_CORES)

    xT = nc.dram_tensor("xT", [C, TOK], F32, kind="ExternalInput")
    wqkv = nc.dram_tensor("wqkv", [C, 3 * 128], F32, kind="ExternalInput")
    bqkv = nc.dram_tensor("bqkv", [3 * 128, 1], F32, kind="ExternalInput")
    wp = nc.dram_tensor("wp", [128, C], F32, kind="ExternalInput")
    bp = nc.dram_tensor("bp", [C, 1], F32, kind="ExternalInput")
    mask_d = nc.dram_tensor("mask", [128, 128], F32, kind="ExternalInput")
    ident_d = nc.dram_tensor("ident2", [128, HD], F32, kind="ExternalInput")
    oT = nc.dram_tensor("oT", [C, TOK], F32, kind="ExternalOutput")

    with tile.TileContext(nc) as tc:
        with (
            tc.tile_pool(name="consts", bufs=1) as consts,
            tc.tile_pool(name="xf", bufs=4) as xfp,
            tc.tile_pool(name="qkvt", bufs=2) as qkvt,
            tc.tile_pool(name="vtok", bufs=2) as vtokp,
            tc.tile_pool(name="pexp", bufs=3) as pexpp,
            tc.tile_pool(name="ytp", bufs=2) as ytp,
            tc.tile_pool(name="otp", bufs=3) as otp,
            tc.tile_pool(name="small", bufs=1) as small,
            tc.tile_pool(name="ps1", bufs=2, space="PSUM") as ps1,
            tc.tile_pool(name="ps2", bufs=2, space="PSUM") as ps2,
            tc.tile_pool(name="ps3", bufs=2, space="PSUM") as ps3,
        ):
            import contextlib

            loop_cm = tc.For_i(0, loop_n, 1) if loop_n > 1 else contextlib.nullcontext()
            # ---- constants ----
            # qkv weights: [C, 384] -> sbuf [128, NKK*384] float32r
            w_sb = consts.tile([128, NKK, 3 * 128], F32R, tag="w_sb")
            wq = wqkv.rearrange("(kk p) m -> kk p m", p=128)
            for kk in range(NKK):
                st = small.tile([128, 3 * 128], F32, tag="wst", name="wst")
                nc.sync.dma_start(out=st, in_=wq[kk])
                nc.vector.tensor_copy(out=w_sb[:, kk, :], in_=st)

            # proj weights [128, C] -> f32r (staged through the same slot)
            wp_sb = consts.tile([128, C], F32R, tag="wp_sb")
            wpr = wp.rearrange("p (hc c) -> p hc c", hc=2)
            for hc in range(2):
                st = small.tile([128, 2 * 256], F32, tag="wst", name="wst")
                nc.sync.dma_start(out=st, in_=wpr[:, hc, :])
                nc.vector.tensor_copy(out=wp_sb[:, 512 * hc : 512 * (hc + 1)], in_=st)

            # biases
            bqkv_sb = consts.tile([128, 3], F32, tag="bqkv_sb")
            bq3 = bqkv.rearrange("(g p) one -> g p one", p=128)
            for g in range(3):
                nc.sync.dma_start(out=bqkv_sb[:, g : g + 1], in_=bq3[g])
            bp_sb = consts.tile([128, 8], F32, tag="bp_sb")
            bp8 = bp.rearrange("(oc p) one -> oc p one", p=128)
            for oc in range(8):
                nc.sync.dma_start(out=bp_sb[:, oc : oc + 1], in_=bp8[oc])

            # causal mask tile (fp32 is fine: only a DVE-mul operand)
            mask_sb = consts.tile([128, 128], F32, tag="mask_sb")
            nc.sync.dma_start(out=mask_sb, in_=mask_d[:, :])

            # ones column (f32r, DVE-rounded)
            ones_f32 = consts.tile([128, 1], F32, tag="ones_f32")
            nc.vector.memset(ones_f32, 1.0)
            ones_r = consts.tile([128, 1], F32R, tag="ones_r")
            nc.vector.tensor_copy(out=ones_r, in_=ones_f32)

            # stacked identity (I64 twice) for PE transpose, f32r
            ident_f32 = consts.tile([128, HD], F32, tag="ident_f32")
            nc.sync.dma_start(out=ident_f32, in_=ident_d[:, :])
            ident_r = consts.tile([128, HD], F32R, tag="ident_r")
            nc.vector.tensor_copy(out=ident_r, in_=ident_f32)

            def emit_proj_chunk(yt_ib, b_p, tb):
                t0p = b_p * T
                for oc in range(8):
                    po = ps1.tile([128, TB], F32, tag="mm512", name="po")
                    nc.tensor.matmul(
                        po,
                        wp_sb[:, 128 * oc : 128 * (oc + 1)],
                        yt_ib,
                        start=True,
                        stop=True,
                    )
                    ot = otp.tile([128, TB], F32, tag="ot", name="ot")
                    if (oc + tb) % 2 == 0:
                        nc.vector.tensor_scalar_add(
                            out=ot, in0=po, scalar1=bp_sb[:, oc : oc + 1]
                        )
                    else:
                        nc.scalar.activation(
                            out=ot, in_=po, func=AF.Identity,
                            bias=bp_sb[:, oc : oc + 1],
                        )
                    nc.sync.dma_start(
                        out=oT[128 * oc : 128 * (oc + 1), t0p + TB * tb : t0p + TB * (tb + 1)],
                        in_=ot,
                    )

            xcache = {}

            def emit_x_load(bb, tb):
                tt0 = bb * T
                xrs = []
                for kk in range(NKK):
                    stg = xfp.tile([128, TB], F32, tag="xst", name="xst", bufs=6)
                    nc.sync.dma_start(
                        out=stg,
                        in_=xT[128 * kk : 128 * (kk + 1), tt0 + TB * tb : tt0 + TB * (tb + 1)],
                    )
                    xr = xfp.tile([128, TB], F32R, tag=f"xr{kk}", name=f"xr{kk}")
                    eng = nc.gpsimd if kk % 2 == 0 else nc.vector
                    eng.tensor_copy(out=xr, in_=stg)
                    xrs.append(xr)
                return xrs

            pending = []
            with loop_cm:
              for b in range(B):
                t0 = b * T  # global token offset of this batch

                # ---- phase A: QKV^T for batch b ----
                qt = qkvt.tile([128, T], F32R, tag="qt")
                kt = qkvt.tile([128, T], F32R, tag="kt")
                vt = qkvt.tile([128, T], F32R, tag="vt")
                dsts = (qt, kt, vt)
                for tb in range(NTB):
                    xrs = xcache.pop((b, tb), None)
                    if xrs is None:
                        xrs = emit_x_load(b, tb)
                    for g in range(3):
                        pqkv = ps1.tile([128, TB], F32, tag="mm512")
                        for kk in range(NKK):
                            nc.tensor.matmul(
                                pqkv,
                                w_sb[:, kk, 128 * g : 128 * (g + 1)],
                                xrs[kk],
                                start=(kk == 0),
                                stop=(kk == NKK - 1),
                            )
                        nc.vector.tensor_scalar_add(
                            out=dsts[g][:, TB * tb : TB * (tb + 1)],
                            in0=pqkv,
                            scalar1=bqkv_sb[:, g : g + 1],
                        )

                # ---- phase B: V token-major [128 tok, 65] tiles ----
                vtok = [vtokp.tile([128, NJT, HD + 1], F32R, tag=f"vtok{h}", name=f"vtok{h}") for h in range(HPC)]
                for h in range(HPC):
                    ones_b = bass.AP(
                        tensor=ones_r.tensor,
                        offset=ones_r.offset,
                        ap=[ones_r.ap[0], [0, NJT], [1, 1]],
                    )
                    nc.vector.tensor_copy(out=vtok[h][:, :, HD : HD + 1], in_=ones_b)
                    for tt in range(NJT):
                        pvt = ps1.tile([128, HD], F32R, tag="mm512")
                        nc.tensor.transpose(
                            pvt,
                            vt[HD * h : HD * (h + 1), 128 * tt : 128 * (tt + 1)],
                            ident_r[HD * h : HD * (h + 1), :],
                        )
                        nc.vector.tensor_copy(out=vtok[h][:, tt, 0:HD], in_=pvt)

                # ---- phase C: causal attention ----
                for ib in range(NIB):
                    yt = ytp.tile([128, TB], F32R, tag=f"yt{ib}", name=f"yt{ib}")
                    if len(pending) >= 3:
                        emit_proj_chunk(*pending.pop(0))
                    if b + 1 < B and (b + 1, ib) not in xcache:
                        xcache[(b + 1, ib)] = emit_x_load(b + 1, ib)
                    i_lo = 512 * ib
                    pys = [ps3.tile([HD + 1, 512], F32, tag="y", name=f"py{hh}") for hh in range(HPC)]
                    njt = 4 * ib + 4
                    for jt in range(njt):
                        coff = max(0, 128 * jt - i_lo)
                        n = 512 - coff
                        pst = ps2.tile([128, 1024], F32, tag="s")
                        for h in range(HPC):
                            nc.tensor.matmul(
                                pst[:, 512 * h + coff : 512 * h + 512],
                                kt[HD * h : HD * (h + 1), 128 * jt : 128 * (jt + 1)],
                                qt[HD * h : HD * (h + 1), i_lo + coff : i_lo + 512],
                                start=True,
                                stop=True,
                                tile_position=(HD * h, 0),
                            )
                        pe = pexpp.tile([128, 1024], F32R, tag="pexp")
                        if coff == 0:
                            nc.scalar.activation(
                                out=pe[:, 0:1024], in_=pst[:, 0:1024], func=AF.Exp, scale=SCALE
                            )
                        else:
                            for h in range(HPC):
                                nc.scalar.activation(
                                    out=pe[:, 512 * h + coff : 512 * h + 512],
                                    in_=pst[:, 512 * h + coff : 512 * h + 512],
                                    func=AF.Exp,
                                    scale=SCALE,
                                )
                        if 128 * jt >= i_lo:  # diagonal tile: triangular mask
                            for h in range(HPC):
                                lo = 512 * h + coff
                                nc.vector.tensor_mul(
                                    pe[:, lo : lo + 128],
                                    pe[:, lo : lo + 128],
                                    mask_sb,
                                )
                        for h in range(HPC):
                            nc.tensor.matmul(
                                pys[h][:, coff:512],
                                vtok[h][:, jt, :],
                                pe[:, 512 * h + coff : 512 * h + 512],
                                start=(jt == 0),
                                stop=(jt == njt - 1),
                            )
                    for h in range(HPC):
                        rs = small.tile([1, 512], F32, tag=f"rs{h}", name=f"rs{h}")
                        nc.vector.reciprocal(out=rs, in_=pys[h][HD : HD + 1, :])
                        rb = small.tile([HD, 512], F32, tag=f"rb{h}", name=f"rb{h}")
                        nc.gpsimd.partition_broadcast(rb, rs)
                        nc.vector.tensor_mul(
                            yt[HD * h : HD * (h + 1), :],
                            pys[h][0:HD, :],
                            rb,
                        )
                    pending.append((yt, b, ib))
              for c in pending:
                  emit_proj_chunk(*c)
              pending.clear()

    nc.compile()
    return nc


def _get_nc():
    global _NC_CACHE
    if _NC_CACHE is None:
        _NC_CACHE = build_kernel()
    return _NC_CACHE


def make_in_maps(x, W_attn, b_attn, W_proj, b_proj):
    x = np.asarray(x, dtype=np.float32)
    W_attn = np.asarray(W_attn, dtype=np.float32)
    b_attn = np.asarray(b_attn, dtype=np.float32)
    W_proj = np.asarray(W_proj, dtype=np.float32)
    b_proj = np.asarray(b_proj, dtype=np.float32)

    xT = np.ascontiguousarray(x.reshape(TOK, C).T)
    mask = np.triu(np.ones((128, 128), dtype=np.float32))  # keep i >= j
    ident2 = np.concatenate([np.eye(HD, dtype=np.float32)] * 2, axis=0)

    in_maps = []
    for c in range(N_CORES):
        h0 = HPC * c
        qs = slice(HD * h0, HD * h0 + 128)
        ks = slice(C + HD * h0, C + HD * h0 + 128)
        vs = slice(2 * C + HD * h0, 2 * C + HD * h0 + 128)
        wqkv_c = np.ascontiguousarray(
            np.concatenate([W_attn[:, qs], W_attn[:, ks], W_attn[:, vs]], axis=1)
        )
        bqkv_c = np.ascontiguousarray(
            np.concatenate([b_attn[qs], b_attn[ks], b_attn[vs]])[:, None]
        )
        wp_c = np.ascontiguousarray(W_proj[128 * c : 128 * (c + 1), :])
        bp_c = (
            b_proj[:, None].astype(np.float32)
            if c == 0
            else np.zeros((C, 1), np.float32)
        )
        in_maps.append(
            {
                "xT": xT,
                "wqkv": wqkv_c,
                "bqkv": bqkv_c,
                "wp": wp_c,
                "bp": np.ascontiguousarray(bp_c),
                "mask": mask,
                "ident2": ident2,
            }
        )
    return in_maps


def gather(res):
    acc = np.zeros((C, TOK), dtype=np.float64)
    for r in res.results:
        acc += r["oT"].astype(np.float64)
    return acc.T.astype(np.float32).reshape(B, T, C)


def kernel(x, W_attn, b_attn, W_proj, b_proj):
    global LAST_RESULTS
    in_maps = make_in_maps(x, W_attn, b_attn, W_proj, b_proj)
    nc = _get_nc()
    res = run_bass_kernel_spmd(nc, in_maps, core_ids=list(range(N_CORES)))
    LAST_RESULTS = res
    return gather(res)
